# revision 88
# baseline (speedup 1.0000x reference)
"""Trainium2 Bass kernel: C = triu(A @ B), A/B upper-triangular 4096x4096 fp32.

Strategy (2D: 4 row-groups x 2 col-groups, SPMD single program, bf16):
  * Core c -> (r = c%4, s = c//4). Core owns row blocks {4j + r, j=0..7}
    (8 slots of 128 rows) and output column tiles q in {2m+s, m=0..3}
    (4 slots of 512 cols).  vs. the 1D row-parallel layout this halves
    the per-core B stream (only its own q columns) at the cost of 2x A.
  * One uniform schedule for all cores: for m-slot and chunk kg (4
    k-tiles of 128), accumulate k in [4j, 8m+7] into psum[j] for every
    active slot j <= 2m+1.  Per-core variation lives entirely in the
    DATA: A^T tiles with k < own-block are zero-filled by the host, and
    B's below-diagonal blocks are zero in the source matrix, so padded
    matmuls contribute exact zeros.  Since A and B are upper-triangular
    the lower triangle of C comes out exactly 0 - no masking needed.
  * bf16 inputs (rel-err ~2e-3 vs 2e-2 budget) halve HBM bytes; C is
    stored bf16 too (rel-err ~4e-3).  Per-core HBM traffic ~16.6 MB,
    PE ~128k columns: balanced at ~53us each.
  * The masked chunk of each m (kg = 2m+1) is the diagonal chunk for
    s=1 cores and all-zero for s=0 cores: loaded with the triangle
    pattern (cols >= 128i of k-row i) and matmul'd width-masked -
    correct for both.  A^T is packed in exact first-use order of the
    emission schedule and streamed just-in-time, one group per chunk.
  * Schedule (model-swept): m order [1,3,2,0] - tiny m=0 last so the
    copy/store drain tail is short; kg descending inside m=2 and m=0,
    which completes psum j exactly at chunk kg=j and spreads the drain.
"""

import numpy as np
from contextlib import ExitStack

import concourse.mybir as mybir
import concourse.tile as tile
from concourse import bacc, bass_utils

N = 4096
P = 128
NCORES = 8
GR = 4             # row groups (cores per column group)
GS = 2             # column groups
NJ = 8             # row-block slots per core (32 blocks / GR)
NM = 4             # 512-wide output column slots per core (8 q-tiles / GS)
QW = 512
NKT = 32           # 128-wide k tiles

# (slot, m) pairs the program computes/writes, in emission order
PAIRS = [(j, m) for m in range(NM) for j in range(2 * m + 2)]
NT = len(PAIRS)    # 20 output tiles of 128x512 per core


def _kgs(m):
    """Chunk order within an m-slot.  Descending puts the k-tiles shared
    by many slots first (more PE work per DMA byte early) and completes
    psum j exactly at chunk kg=j, spreading the copy/store drain across
    the whole m instead of bunching it at the end."""
    r = range(2 * m + 2)
    return list(reversed(r)) if m in KG_DESC else list(r)


def _a_layout(seq):
    """A^T tiles in exact first-use order of the emission schedule; group
    g holds the tiles first needed by chunk g, so A streams just-in-time
    interleaved with the B chunks.  j-outer within a chunk so the tiles
    of each (j, k-pair) are adjacent (DoubleRow fuses pairs)."""
    idx = {}
    groups = []
    t = 0
    for m, kg in seq:
        t0 = t
        for j in range(2 * m + 2):
            if kg < j:
                continue       # k = 4kg+i >= 4j is all-or-nothing per chunk
            for i in range(4):
                k = 4 * kg + i
                if (j, k) not in idx:
                    idx[(j, k)] = t
                    t += 1
        groups.append((t0, t))
    return idx, groups


def _flags(seq):
    """First/last processed k per (m, j) and out-emission points, derived
    from the chunk sequence + within-chunk matmul emission order (j-major
    in the masked chunk, i-major elsewhere).  The first-processed matmul
    of any (m, j) always lands at i=0 of its chunk, so the c0-masked
    psum region is initialized full-width."""
    firstk, lastk = {}, {}
    for m, kg in seq:
        act = range(2 * m + 2)
        masked = kg == 2 * m + 1
        for j in (act if masked else [None]):
            for i in range(4):
                k = 4 * kg + i
                for jj in ([j] if masked else act):
                    if k < 4 * jj:
                        continue
                    firstk.setdefault((m, jj), k)
                    lastk[(m, jj)] = k
    # out-emission point: the position of the chunk holding (m, j)'s
    # last matmul
    outs = {p: [] for p in range(len(seq))}
    lastpos = {}
    for p, (m, kg) in enumerate(seq):
        for j in range(2 * m + 2):
            ks = [4 * kg + i for i in range(4) if 4 * kg + i >= 4 * j]
            if ks and lastk[(m, j)] in ks:
                lastpos[(m, j)] = p
    for (m, j), p in lastpos.items():
        outs[p].append(j)
    return firstk, lastk, outs


def set_order(order, kg_desc=None, seq=None):
    """Set the m emission order and recompute the derived layout.  seq
    overrides the chunk sequence (any interleaving containing every
    (m, kg) exactly once; within-m order is free)."""
    global M_ORDER, ATIDX, AGROUPS, ATOT, DRAIN_MS, KG_DESC, CHUNK_SEQ
    global FIRSTK, LASTK, OUTS
    if kg_desc is not None:
        KG_DESC = set(kg_desc)
    M_ORDER = list(order)
    CHUNK_SEQ = (list(seq) if seq is not None else
                 [(m, kg) for m in M_ORDER for kg in _kgs(m)])
    ATIDX, AGROUPS = _a_layout(CHUNK_SEQ)
    ATOT = max(t1 for _, t1 in AGROUPS)    # 144 packed A tiles per core
    FIRSTK, LASTK, OUTS = _flags(CHUNK_SEQ)
    tail_ms = list(dict.fromkeys(m for m, _ in reversed(CHUNK_SEQ)))
    DRAIN_MS = set(tail_ms[:2])
    _nc_cache.clear()


_nc_cache = {}
KG_DESC = set()
# chunk sequence found by randomized local search over adjacent swaps
# (feasible = concurrently-open m's fit in 8 psum banks): m1 ascending,
# m3 scrambled, m0 interleaved into m2's window, drain spread
set_order([1, 3, 2, 0], kg_desc={0, 2}, seq=[
    (1, 0), (1, 1), (1, 2), (1, 3),
    (3, 3), (3, 2), (3, 0), (3, 4), (3, 1), (3, 7), (3, 6), (3, 5),
    (0, 0), (2, 2), (0, 1), (2, 3), (2, 5), (2, 4), (2, 1), (2, 0),
])

# matmul dtype mode: "bf16" (single pass, ~8-bit mantissa, half the HBM
# bytes), "fp32r" (~11-bit mantissa, 4x PE cost at width<256), "fp32"
# (exact, 4x slower PE)
MODE = "bf16"
C_BF16 = True      # store C as bf16 (halves output traffic)

# pool buffer counts (double/triple buffering)
BUFS_B = 6
BUFS_O = 6
BUFS_PS = 8

# drain engine assignment, cycled per tile: copy 0=DVE 1=ACT(scalar.copy),
# store 0=ACT ring 1=SP ring
DRAIN_COPY = [0]
DRAIN_STORE = [0, 1]

# load masked chunks as one full rectangle (below-diagonal region of the
# packed B is naturally zero) instead of 4 staircase DMAs
MASKED_FULL_LOAD = False

# split A groups with at least this many tiles into two (or three) DMAs
# so early matmuls of the chunk wait on a fraction of the bytes
A_SPLIT = 24
A_SPLIT3 = 999
A_CUT_FRAC = 0.5   # position of the first cut within a split A group

# chunk-sequence positions whose full B chunk loads as two halves
# (model-swept local optimum for the supply-bound early window)
B_SPLIT_POS = {2, 3, 14}
B_SPLIT_CUT = 2    # first sub-chunk holds this many k-tiles

# split the very last tile's copy/store into halves so the second store
# overlaps the first's DMA-pipeline latency (stores on parallel rings)
FINAL_SPLIT = False

# masked-chunk positions loaded as one full rectangle (1 DMA instead of
# 4 staircase slices; the extra below-diagonal bytes are natural zeros)
MFL_POS = set()




def build_nc(mode=MODE, rep=1, variant="full"):
    """rep>1 repeats the whole compute (for dispatch-overhead-cancelling
    timing).  variant: "full" | "nomm" (DMAs only) | "nodma" (matmuls
    only)."""
    if (mode, rep, variant) in _nc_cache:
        return _nc_cache[(mode, rep, variant)]
    dt_in = {
        "fp8dr": mybir.dt.float8e4,
        "bf16": mybir.dt.bfloat16,
        "fp32r": mybir.dt.float32r,
        "fp32": mybir.dt.float32,
    }[mode]
    dt_c = mybir.dt.bfloat16 if C_BF16 else mybir.dt.float32
    # DoubleRow fuses k-tile pairs at 0.5 cycles/row; masked chunks load
    # the full rectangle (below-diagonal data is naturally zero)
    dr = mode == "fp8dr"

    nc = bacc.Bacc("TRN2", target_bir_lowering=False, debug=False,
                   num_devices=NCORES)
    # partition-major packed layouts (see pack_inputs): per-partition data
    # is contiguous so every DMA is 128 descriptors of large runs.
    # Apack row = p(k-within-tile), col = t*P + m  (first-use tile order)
    a_dram = nc.dram_tensor("Apack", [P, ATOT * P], dt_in,
                            kind="ExternalInput").ap()
    # B row = m*P + p, col = k*QW + n   (core's q column = 2m + s)
    b_dram = nc.dram_tensor("B", [NM * P, NKT * QW], dt_in,
                            kind="ExternalInput").ap()
    c_dram = nc.dram_tensor("Cout", [NT * P, QW], dt_c,
                            kind="ExternalOutput").ap()

    with tile.TileContext(nc) as tc:
        with ExitStack() as ctx:
            apool = ctx.enter_context(tc.tile_pool(name="apool", bufs=1))
            bpool = ctx.enter_context(tc.tile_pool(name="bpool", bufs=BUFS_B))
            opool = ctx.enter_context(tc.tile_pool(name="opool", bufs=BUFS_O))
            pspool = ctx.enter_context(
                tc.tile_pool(name="pspool", bufs=BUFS_PS, space="PSUM"))

            do_bdma = variant in ("full", "nomm")
            do_mm = variant in ("full", "nodma")
            do_copy = variant in ("full", "nomm", "nodma")
            do_store = variant in ("full", "nomm", "nodma")

            a_sb = apool.tile([P, ATOT, P], dt_in)

            def _load_a_group(g, ring=None):
                ring = ring or nc.sync
                t0, t1 = AGROUPS[g]
                if t0 == t1:
                    return
                n = t1 - t0
                cuts = [t0, t1]
                if n >= A_SPLIT3:
                    cuts = [t0, t0 + n // 3, t0 + 2 * n // 3, t1]
                elif n >= A_SPLIT:
                    if A_CUT_FRAC == 0.5:
                        cuts = [t0, (t0 + t1) // 2, t1]
                    else:
                        c = max(4, (int(n * A_CUT_FRAC) + 3) // 4 * 4)
                        cuts = [t0, min(t0 + c, t1), t1]
                for lo, hi in zip(cuts, cuts[1:]):
                    ring.dma_start(
                        a_sb[:, lo:hi, :],
                        a_dram[:, lo * P:hi * P].rearrange(
                            "p (t m) -> p t m", m=P))

            def _load_chunk(bt, m, kg, ring=None, split=False, full=False):
                ring = ring or nc.sync
                if kg == 2 * m + 1 and not (full or MASKED_FULL_LOAD or dr):
                    # masked chunk: diagonal for s=1, all-zero for s=0 -
                    # per k-row load only cols >= 128i (the rest of the
                    # tile is stale and masked out of the matmuls)
                    for i in range(4):
                        col = (4 * kg + i) * QW + 128 * i
                        ring.dma_start(
                            bt[:, i, 128 * i:],
                            b_dram[m * P:(m + 1) * P,
                                   col:col + QW - 128 * i])
                elif split:
                    # two sub-chunk DMAs so early-k matmuls gate on a
                    # fraction of the bytes (supply-critical windows only)
                    c = B_SPLIT_CUT
                    for h0, h1 in ((0, c), (c, 4)):
                        col = (4 * kg + h0) * QW
                        ring.dma_start(
                            bt[:, h0:h1, :],
                            b_dram[m * P:(m + 1) * P,
                                   col:col + (h1 - h0) * QW].rearrange(
                                       "p (ko n) -> p ko n", ko=h1 - h0))
                else:
                    ring.dma_start(
                        bt[:],
                        b_dram[m * P:(m + 1) * P,
                               4 * kg * QW:(4 * kg + 4) * QW].rearrange(
                                   "p (ko n) -> p ko n", ko=4))

            def _emit_out(m, j, nj, psums):
                # copy psum j to SBUF (dtype convert) and store the tile
                drain = m in DRAIN_MS
                t = PAIRS.index((j, m))
                ot = opool.tile([P, QW], dt_c, tag="ot")
                if (FINAL_SPLIT and do_mm
                        and (m, LASTK[(m, j)] // 4) == CHUNK_SEQ[-1]):
                    h = QW // 2
                    rows = c_dram[t * P:(t + 1) * P, :]
                    nc.vector.tensor_copy(ot[:, :h], psums[j][:, :h])
                    if do_store:
                        nc.scalar.dma_start(rows[:, :h], ot[:, :h])
                    nc.vector.tensor_copy(ot[:, h:], psums[j][:, h:])
                    if do_store:
                        nc.sync.dma_start(rows[:, h:], ot[:, h:])
                    return
                if do_mm:
                    # in the drain, optionally spread copies over DVE+ACT
                    # so the tail isn't a serial DVE chain (GPSIMD cannot
                    # read PSUM)
                    if drain and DRAIN_COPY[nj % len(DRAIN_COPY)]:
                        nc.scalar.copy(ot[:], psums[j][:])
                    else:
                        nc.vector.tensor_copy(ot[:], psums[j][:])
                else:
                    src = a_sb[:, 4 * j:4 * j + 4, :]
                    if dt_in == mybir.dt.float32r:
                        src = src.bitcast(mybir.dt.float32)
                    nc.vector.tensor_copy(
                        ot[:].rearrange("p (a b) -> p a b", a=4), src)
                if do_store:
                    # ACT HWDGE ring keeps compute-gated stores out of the
                    # B-stream's SP FIFO; in the drain the B stream is
                    # done, so SP is free too
                    ring = (nc.sync if drain
                            and DRAIN_STORE[nj % len(DRAIN_STORE)]
                            else nc.scalar)
                    ring.dma_start(c_dram[t * P:(t + 1) * P, :], ot[:])

            bt_fixed = None
            for _r in range(rep):
                mpsums = {}
                ndone = {m: 0 for m in M_ORDER}
                for g, (m, kg) in enumerate(CHUNK_SEQ):
                    act = list(range(2 * m + 2))
                    if do_mm and m not in mpsums:
                        mpsums[m] = {
                            j: pspool.tile([P, QW], mybir.dt.float32,
                                           tag="ps", name=f"ps_{_r}_{m}_{j}")
                            for j in act
                        }
                    psums = mpsums.get(m, {})
                    if _r == 0:
                        _load_a_group(g)
                    if do_bdma:
                        bt = bpool.tile([P, 4, QW], dt_in, tag="bt")
                        _load_chunk(bt, m, kg, split=g in B_SPLIT_POS,
                                    full=g in MFL_POS)
                    elif do_mm:
                        if bt_fixed is None:
                            bt_fixed = bpool.tile([P, 4, QW], dt_in,
                                                  tag="bt", name="bt_fixed")
                            _load_chunk(bt_fixed, 0, 0)
                        bt = bt_fixed
                    if do_mm and (do_bdma or bt_fixed is not None):
                        masked = kg == 2 * m + 1
                        if dr:
                            # DoubleRow: fuse the (i0,i1) and (i2,i3)
                            # k-tile pairs; on the masked chunk the second
                            # pair only touches cols >= 256 (data below is
                            # zero); j-major so psum stops stagger
                            for j in act:
                                if kg < j:
                                    continue
                                for u in (0, 1):
                                    k0 = 4 * kg + 2 * u
                                    c0 = 256 * u if masked else 0
                                    t0 = ATIDX[(j, k0)]
                                    nc.tensor.matmul(
                                        psums[j][:, c0:],
                                        a_sb[:, t0:t0 + 2, :],
                                        bt[:, 2 * u:2 * u + 2, c0:],
                                        start=k0 == FIRSTK[(m, j)],
                                        stop=k0 + 1 == LASTK[(m, j)],
                                        perf_mode=(
                                            mybir.MatmulPerfMode.DoubleRow))
                            j = None
                        else:
                            # j-major within the masked chunk so psum
                            # stops/starts stagger
                            for j in (act if masked else [None]):
                                for i in range(4):
                                    k = 4 * kg + i
                                    for jj in ([j] if masked else act):
                                        if k < 4 * jj:
                                            continue
                                        c0 = 128 * i if masked else 0
                                        nc.tensor.matmul(
                                            psums[jj][:, c0:],
                                            a_sb[:, ATIDX[(jj, k)], :],
                                            bt[:, i, c0:],
                                            start=k == FIRSTK[(m, jj)],
                                            stop=k == LASTK[(m, jj)])
                    if not (do_copy or do_store):
                        continue
                    if do_mm and not (do_bdma or bt_fixed is not None):
                        continue
                    for j in OUTS[g]:
                        _emit_out(m, j, ndone[m], psums)
                        ndone[m] += 1
    nc.compile()
    _nc_cache[(mode, rep, variant)] = nc
    return nc


def pack_inputs(A, B, mode=MODE):
    """Build per-core in_maps (partition-major packed layouts)."""
    import ml_dtypes
    A = np.ascontiguousarray(np.asarray(A, dtype=np.float32))
    B = np.ascontiguousarray(np.asarray(B, dtype=np.float32))
    dt_np = {
        "fp8dr": ml_dtypes.float8_e4m3,
        "bf16": ml_dtypes.bfloat16,
        "fp32r": np.float32,
        "fp32": np.float32,
    }[mode]

    # B[128k+p, 512q+n] -> per col-group s: Bp[m, p, k, n] with q = 2m+s
    b_packs = []
    b4 = B.reshape(NKT, P, GS * NM, QW)
    for s in range(GS):
        qsel = [2 * m + s for m in range(NM)]
        b_packs.append(np.ascontiguousarray(
            b4[:, :, qsel, :].transpose(2, 1, 0, 3).astype(dt_np)
        ).reshape(NM * P, NKT * QW))

    in_maps = []
    for c in range(NCORES):
        r, s = c % GR, c // GR
        ap = np.zeros((ATOT, P, P), np.float32)
        for (j, k), t in ATIDX.items():
            b = GR * j + r
            if k >= b:
                ap[t] = A[P * b:P * b + P, P * k:P * k + P].T
        # [t, p, m] -> [p, t, m] -> [P, ATOT*P]
        apk = np.ascontiguousarray(
            ap.astype(dt_np).transpose(1, 0, 2)).reshape(P, ATOT * P)
        in_maps.append({"Apack": apk, "B": b_packs[s]})
    return in_maps


def unpack_output(results):
    C = np.zeros((N, N), np.float32)
    for c, r in enumerate(results):
        rr, s = c % GR, c // GR
        co = np.asarray(r["Cout"]).astype(np.float32).reshape(NT, P, QW)
        for t, (j, m) in enumerate(PAIRS):
            b = GR * j + rr
            q = 2 * m + s
            if P * b >= QW * (q + 1):
                continue               # fully below-diagonal tile: zeros
            C[P * b:P * b + P, QW * q:QW * q + QW] = co[t]
    return C


def kernel(A, B):
    nc = build_nc(MODE)
    in_maps = pack_inputs(A, B, MODE)
    res = bass_utils.run_bass_kernel_spmd(
        nc, in_maps, core_ids=list(range(NCORES)), trace=False)
    return unpack_output(res.results)
